# revision 18
# baseline (speedup 1.0000x reference)
"""Talking-heads attention with GFSA reaction term — TRN2 Bass kernel, 8 cores.

COLLECTIVE-FREE design. Sharding: (batch b, query-half) -> core c = b*2 + half.
The m/key axis is ROTATED on the host so rows 0..511 are the core's own query
rows and 512..1023 the partner's. Each core redundantly computes the full-N
attention probabilities E and the full-N pass-1 product w1 = attn3 @ v for its
batch (both halves), so there is NO cross-core exchange: every core's NEFF runs
independently and launch skew between cores cannot inflate exec time.

Math (associativity rewrite — never materialize attn3 @ attn3):
  attn2[g]  = sum_h W1[g,h] (q_h*SCALE) @ k_h^T + b1[g]   (folded into QK^T)
  s_g       = softmax_m(attn2[g])  (E=exp stored fp16 [m,n] layout, normalized
                                    in place by broadcast 1/Z; no max-subtract
                                    needed: |scores| <~ 6)
  attn3[g'] = sum_g W2[g',g] s_g + b2[g']                  (folded into A@v)
  w1[g']    = attn3[g'] @ v_g'                              (ALL 1024 rows)
  u         = (1-2*lam)v + 3*lam*w1  (+ b2 colsum terms)
  out[g']   = attn3[g'][own rows] @ u_g'
  y         = out @ Wo^T + ob

Per-g software pipeline in the score loop: while the ACT/DVE engines exp and
normalize head g's probabilities, the PE runs the pass-1(partner) matmuls of
head g-1, so the softmax normalization latency never stalls the PE.
"""
import numpy as np

import concourse.bacc as bacc
import concourse.mybir as mybir
import concourse.tile as tile
from concourse.bass_utils import run_bass_kernel_spmd
from concourse.masks import make_identity

B, N, DIM, HEADS, HD = 4, 1024, 768, 12, 64
NH = N // 2                      # query rows per core (own half)
SCALE = HD ** -0.5
f32 = mybir.dt.float32
f16 = mybir.dt.float16
f8 = mybir.dt.float8e4
DRMODE = mybir.MatmulPerfMode.DoubleRow
AL = mybir.AluOpType
AF = mybir.ActivationFunctionType

TRACE = False                    # test.py may flip this for profiling
TRACE_KW = {}
DEBUG = False                    # dump intermediates as extra outputs


def _build():
    nc = bacc.Bacc("TRN2", target_bir_lowering=False, debug=False, num_devices=8)

    xf_T = nc.declare_dram_parameter("xf_T", [DIM, N], f16, isOutput=False)
    wq_T = nc.declare_dram_parameter("wq_T", [DIM, DIM], f16, isOutput=False)
    wk_T = nc.declare_dram_parameter("wk_T", [DIM, DIM], f16, isOutput=False)
    wv_T = nc.declare_dram_parameter("wv_T", [DIM, DIM], f16, isOutput=False)
    wo_T = nc.declare_dram_parameter("wo_T", [DIM, DIM], f16, isOutput=False)
    w1v = nc.declare_dram_parameter("w1v", [128, 72], f32, isOutput=False)
    b1bc = nc.declare_dram_parameter("b1bc", [128, HEADS], f32, isOutput=False)
    w2f = nc.declare_dram_parameter("w2f", [1, HEADS * DIM], f16, isOutput=False)
    uc1 = nc.declare_dram_parameter("uc1", [1, DIM], f16, isOutput=False)
    uc2 = nc.declare_dram_parameter("uc2", [1, DIM], f16, isOutput=False)
    b2blk = nc.declare_dram_parameter("b2blk", [1, DIM], f16, isOutput=False)
    ob = nc.declare_dram_parameter("ob", [1, DIM], f16, isOutput=False)
    y = nc.declare_dram_parameter("y", [NH, DIM], f32, isOutput=True)
    if DEBUG:
        dbg_E = nc.declare_dram_parameter("dbg_E", [128, 8, HEADS, NH], f16,
                                          isOutput=True)
        dbg_qT = nc.declare_dram_parameter("dbg_qT", [128, 6, N], f16, isOutput=True)
        dbg_kT = nc.declare_dram_parameter("dbg_kT", [128, 6, N], f16, isOutput=True)
        dbg_v = nc.declare_dram_parameter("dbg_v", [128, 8, DIM], f16, isOutput=True)
        dbg_w1p = nc.declare_dram_parameter("dbg_w1p", [128, 4, 512], f16,
                                            isOutput=True)
        dbg_w1pb = nc.declare_dram_parameter("dbg_w1pb", [128, 4, 256], f32,
                                             isOutput=True)
        dbg_u = nc.declare_dram_parameter("dbg_u", [128, 8, DIM], f16, isOutput=True)
        dbg_acc = nc.declare_dram_parameter("dbg_acc", [128, 4, DIM], f32,
                                            isOutput=True)
        dbg_b2v = nc.declare_dram_parameter("dbg_b2v", [128, DIM], f32, isOutput=True)
        dbg_ucb = nc.declare_dram_parameter("dbg_ucb", [128, DIM], f16, isOutput=True)
        dbg_w1o = nc.declare_dram_parameter("dbg_w1o", [128, 4, DIM], f16,
                                            isOutput=True)
        dbg_Ep = nc.declare_dram_parameter("dbg_Ep", [128, 8, NH], f16, isOutput=True)

    with tile.TileContext(nc) as tc:
        with tc.tile_pool(name="persist", bufs=1) as pp, \
             tc.tile_pool(name="w2p", bufs=2) as w2p:
            # [m%128, m//128, g, n_own] fp16 probabilities (own-n columns);
            # written as exp(scores), then normalized in place.
            E = pp.tile([128, 8, HEADS, NH], f16)
            v16 = pp.tile([128, 8, DIM], f16)          # [m%128, m//128, (g',d)]
            w1p16 = pp.tile([128, 4, 512], f16)        # staged partner w1 (c<512)
            accB = pp.tile([128, 4, 256], f32)         # partner w1 (c 512:768)
            w1v_sb = pp.tile([128, 72], f32)
            b1_sb = pp.tile([128, HEADS], f32)
            b2bc = pp.tile([128, DIM], f16)
            ones128 = pp.tile([128, 128], f16)
            b2v = pp.tile([128, DIM], f32)
            nc.sync.dma_start(w1v_sb[:], w1v[:])
            nc.sync.dma_start(b1_sb[:], b1bc[:])
            nc.gpsimd.dma_start(b2bc[:], b2blk[0:1, :].to_broadcast((128, DIM)))
            nc.vector.memset(ones128[:], 1.0)

            def load_w2bc(g):
                w2bc = w2p.tile([128, DIM], f16, tag="w2bc")
                nc.gpsimd.dma_start(
                    w2bc[:], w2f[0:1, g * DIM:(g + 1) * DIM].to_broadcast((128, DIM)))
                return w2bc

            with tc.tile_pool(name="qk", bufs=1) as qk:
                qT = qk.tile([128, 6, N], f16)         # [d%128, d//128, n]
                kT = qk.tile([128, 6, N], f16)         # [d%128, d//128, m]

                # ---- Phase A: QKV projections (fp16 weights) ----------------
                with tc.tile_pool(name="pha", bufs=1) as pa, \
                     tc.tile_pool(name="psa", bufs=2, space="PSUM") as psa, \
                     tc.tile_pool(name="psav", bufs=1, space="PSUM") as psav:
                    xf = pa.tile([128, 6, N], f16)
                    wq_sb = pa.tile([128, 6, DIM], f16)
                    wk_sb = pa.tile([128, 6, DIM], f16)
                    wv_sb = pa.tile([128, 6, DIM], f16)
                    xfr = xf_T.rearrange("(c p) n -> p c n", p=128)
                    wqr = wq_T.rearrange("(c p) n -> p c n", p=128)
                    wkr = wk_T.rearrange("(c p) n -> p c n", p=128)
                    wvr = wv_T.rearrange("(c p) n -> p c n", p=128)
                    for d in range(6):
                        nc.sync.dma_start(xf[:, d, :], xfr[:, d, :])
                        nc.scalar.dma_start(wq_sb[:, d, :], wqr[:, d, :])
                    for d in range(6):
                        nc.sync.dma_start(wk_sb[:, d, :], wkr[:, d, :])
                        nc.scalar.dma_start(wv_sb[:, d, :], wvr[:, d, :])

                    for qc in range(6):                # qT[c, n] = sum_d wq[d,c]x[n,d]
                        ps = psa.tile([128, N], f32, tag="psqk")
                        for d in range(6):
                            for h in range(2):         # fp16 moving operand <= 512
                                nc.tensor.matmul(ps[:, h * NH:(h + 1) * NH],
                                                 wq_sb[:, d, qc * 128:(qc + 1) * 128],
                                                 xf[:, d, h * NH:(h + 1) * NH],
                                                 start=(d == 0), stop=(d == 5))
                        nc.scalar.copy(qT[:, qc, :], ps[:])
                    for kc in range(6):
                        ps = psa.tile([128, N], f32, tag="psqk")
                        for d in range(6):
                            for h in range(2):
                                nc.tensor.matmul(ps[:, h * NH:(h + 1) * NH],
                                                 wk_sb[:, d, kc * 128:(kc + 1) * 128],
                                                 xf[:, d, h * NH:(h + 1) * NH],
                                                 start=(d == 0), stop=(d == 5))
                        nc.scalar.copy(kT[:, kc, :], ps[:])
                    for mt in range(8):                # v[m, c] = sum_d x[m,d]wv[d,c]
                        ps = psa.tile([128, N], f32, tag="psqk")
                        for d in range(6):
                            nc.tensor.matmul(ps[:, 0:512],
                                             xf[:, d, mt * 128:(mt + 1) * 128],
                                             wv_sb[:, d, 0:512],
                                             start=(d == 0), stop=(d == 5))
                            nc.tensor.matmul(ps[:, 512:768],
                                             xf[:, d, mt * 128:(mt + 1) * 128],
                                             wv_sb[:, d, 512:768],
                                             start=(d == 0), stop=(d == 5))
                        nc.scalar.copy(v16[:, mt, :], ps[:, 0:DIM])
                    # b2v = b2blk * colsum(v), row-replicated via all-ones lhsT
                    psVa = psav.tile([128, 512], f32)
                    psVb = psav.tile([128, 256], f32)
                    for mt in range(8):
                        nc.tensor.matmul(psVa[:], ones128[:], v16[:, mt, 0:512],
                                         start=(mt == 0), stop=(mt == 7))
                        nc.tensor.matmul(psVb[:], ones128[:], v16[:, mt, 512:768],
                                         start=(mt == 0), stop=(mt == 7))
                    nc.vector.tensor_tensor(b2v[:, 0:512], psVa[:], b2bc[:, 0:512],
                                            AL.mult)
                    nc.vector.tensor_tensor(b2v[:, 512:768], psVb[:], b2bc[:, 512:768],
                                            AL.mult)

                # ---- Phase B + pass-1(partner): per-g pipeline --------------
                with tc.tile_pool(name="qsc", bufs=1) as qscp, \
                     tc.tile_pool(name="ep", bufs=2) as epp, \
                     tc.tile_pool(name="zp", bufs=1) as zpp, \
                     tc.tile_pool(name="vt", bufs=2) as vtp, \
                     tc.tile_pool(name="psb", bufs=2, space="PSUM") as psb, \
                     tc.tile_pool(name="btp", bufs=2, space="PSUM") as btp, \
                     tc.tile_pool(name="psw", bufs=1, space="PSUM") as psw:
                    psWa = [psw.tile([128, 512], f32, name=f"psWa{i}") for i in range(4)]
                    nc.vector.memset(accB[:], 0.0)
                    prev = None        # (E_part, Vt) of g-1
                    for g in range(HEADS + 1):
                        if g < HEADS:
                            qsc = qscp.tile([128, 6, N], f16, tag="qsc")
                            for i in range(6):
                                nc.vector.tensor_scalar(
                                    qsc[:, i, :], qT[:, i, :],
                                    w1v_sb[:, g * 6 + i:g * 6 + i + 1], None, AL.mult)
                            E_part = epp.tile([128, 8, NH], f16, tag="ep")
                            E_part8 = epp.tile([128, 8, NH], f8, tag="ep8")
                            for mt in range(8):
                                for nh in range(2):
                                    ps = psb.tile([128, NH], f32, tag="ps")
                                    for i in range(6):
                                        nc.tensor.matmul(
                                            ps[:], kT[:, i, mt * 128:(mt + 1) * 128],
                                            qsc[:, i, nh * NH:(nh + 1) * NH],
                                            start=(i == 0), stop=(i == 5))
                                    dst = E[:, mt, g, :] if nh == 0 else E_part[:, mt, :]
                                    nc.scalar.activation(dst, ps[:], AF.Exp,
                                                         bias=b1_sb[:, g:g + 1],
                                                         scale=1.0)
                            cur = (E_part8, g)
                        # pass-1 partner matmuls for g-1: PE fills the softmax
                        # normalization latency of head g with these.
                        if prev is not None:
                            E_p, pg = prev
                            Vt_p = vts[pg % 2]
                            for ns in range(4):
                                bt = btp.tile([128, 256], f32, tag="bt")
                                for t in range(4):
                                    lhs = E_p[:, 2 * t:2 * t + 2,
                                              ns * 128:(ns + 1) * 128]
                                    first = (pg == 0 and t == 0)
                                    last = (pg == HEADS - 1 and t == 3)
                                    nc.tensor.matmul(psWa[ns][:], lhs,
                                                     Vt_p[:, 2 * t:2 * t + 2, 0:512],
                                                     start=first, stop=last,
                                                     perf_mode=DRMODE)
                                    nc.tensor.matmul(bt[:], lhs,
                                                     Vt_p[:, 2 * t:2 * t + 2, 512:768],
                                                     start=(t == 0), stop=(t == 3),
                                                     perf_mode=DRMODE)
                                nc.vector.tensor_add(accB[:, ns, :], accB[:, ns, :],
                                                     bt[:])
                        if g < HEADS:
                            # Z row sums: DVE pre-sum over mt, then a 1-col
                            # ones matmul over partitions, recip + broadcast.
                            zp = zpp.tile([128, N], f16, tag="zp")
                            nc.vector.tensor_add(zp[:, 0:NH], E[:, 0, g, :],
                                                 E[:, 1, g, :])
                            nc.vector.tensor_add(zp[:, NH:N], E_part[:, 0, :],
                                                 E_part[:, 1, :])
                            for mt in range(2, 8):
                                nc.vector.tensor_add(zp[:, 0:NH], zp[:, 0:NH],
                                                     E[:, mt, g, :])
                                nc.vector.tensor_add(zp[:, NH:N], zp[:, NH:N],
                                                     E_part[:, mt, :])
                            psZo = psb.tile([1, NH], f32, tag="ps")
                            nc.tensor.matmul(psZo[0:1, :], ones128[:, 0:1],
                                             zp[:, 0:NH], start=True, stop=True)
                            psZp = psb.tile([1, NH], f32, tag="ps")
                            nc.tensor.matmul(psZp[0:1, :], ones128[:, 0:1],
                                             zp[:, NH:N], start=True, stop=True)
                            zsb = zpp.tile([1, N], f16, tag="zsb")
                            with nc.allow_low_precision(
                                    reason="Z fits fp16; validated vs ref"):
                                nc.scalar.copy(zsb[0:1, 0:NH], psZo[0:1, :])
                                nc.scalar.copy(zsb[0:1, NH:N], psZp[0:1, :])
                            with nc.allow_low_precision(
                                    reason="1/Z fits fp16; validated vs ref"):
                                nc.vector.reciprocal(zsb[0:1, 0:NH], zsb[0:1, 0:NH])
                                nc.vector.reciprocal(zsb[0:1, NH:N], zsb[0:1, NH:N])
                            zb = zpp.tile([128, N], f16, tag="zb")
                            nc.gpsimd.partition_broadcast(zb[:, :], zsb[0:1, :])
                            with nc.allow_low_precision(
                                    reason="partner probs fp8; validated vs ref"):
                                for mt in range(8):
                                    nc.vector.tensor_tensor(E[:, mt, g, :],
                                                            E[:, mt, g, :],
                                                            zb[:, 0:NH], AL.mult)
                                    nc.vector.tensor_tensor(E_part8[:, mt, :],
                                                            E_part[:, mt, :],
                                                            zb[:, NH:N], AL.mult)
                            w2bc = load_w2bc(g)
                            Vt = vtp.tile([128, 8, DIM], f8, tag="vt",
                                          name=f"vt{g % 2}")
                            if g == 0:
                                vts = [None, None]
                            vts[g % 2] = Vt
                            with nc.allow_low_precision(
                                    reason="partner pass-1 fp8; validated vs ref"):
                                for mt in range(8):
                                    nc.vector.tensor_tensor(Vt[:, mt, :],
                                                            v16[:, mt, :],
                                                            w2bc[:], AL.mult)
                            prev = cur

                    # stage partner-half w1 to fp16 SBUF (psw pool closes here)
                    for j in range(4):
                        nc.scalar.copy(w1p16[:, j, :], psWa[j][:])
                    if DEBUG:
                        pass
                        nc.sync.dma_start(dbg_E[:], E[:])
                        nc.sync.dma_start(dbg_w1p[:], w1p16[:])
                        nc.sync.dma_start(dbg_w1pb[:], accB[:])
                        nc.sync.dma_start(dbg_qT[:], qT[:])
                        nc.sync.dma_start(dbg_kT[:], kT[:])
                        nc.sync.dma_start(dbg_v[:], v16[:])

            # qT/kT and all phase-B pools freed here.
            with tc.tile_pool(name="late", bufs=1) as late, \
                 tc.tile_pool(name="vt2", bufs=2) as vtp2:
                u16 = late.tile([128, 8, DIM], f16)
                uc1bc = late.tile([128, DIM], f16)
                uc2bc = late.tile([128, DIM], f16)
                ucb = late.tile([128, DIM], f16)
                b2u = late.tile([128, DIM], f32)
                acc = late.tile([128, 4, DIM], f32)    # pass-2 accumulator
                nc.gpsimd.dma_start(uc1bc[:], uc1[0:1, :].to_broadcast((128, DIM)))
                nc.gpsimd.dma_start(uc2bc[:], uc2[0:1, :].to_broadcast((128, DIM)))
                nc.vector.tensor_tensor(ucb[:], b2v[:], uc2bc[:], AL.mult)

                # u = uc1*v + uc2*w1 + ucb
                def build_u(j, wa, wb):
                    t1 = late.tile([128, DIM], f16, tag="ub1")
                    t2 = late.tile([128, DIM], f16, tag="ub2")
                    nc.vector.tensor_tensor(t1[:], v16[:, j, :], uc1bc[:], AL.mult)
                    nc.vector.tensor_tensor(t2[:, 0:512], wa, uc2bc[:, 0:512], AL.mult)
                    nc.vector.tensor_tensor(t2[:, 512:768], wb, uc2bc[:, 512:768],
                                            AL.mult)
                    nc.vector.tensor_add(t1[:], t1[:], t2[:])
                    nc.vector.tensor_add(u16[:, j, :], t1[:], ucb[:])

                # ---- pass-1 own half: w1 for m-tiles 0..3 -------------------
                with tc.tile_pool(name="pso", bufs=1, space="PSUM") as pso:
                    psOa = [pso.tile([128, 512], f32, name=f"psOa{i}") for i in range(4)]
                    psOb = [pso.tile([128, 256], f32, name=f"psOb{i}") for i in range(4)]
                    for g in range(HEADS):
                        if g == 1:
                            # partner u tiles: emitted late so the DVE builds
                            # Vt2(0) first and the PE transition stall is short
                            for j in range(4):
                                build_u(4 + j, w1p16[:, j, :], accB[:, j, :])
                        w2bc = load_w2bc(g)
                        Vt = vtp2.tile([128, 8, DIM], f16, tag="vt2")
                        for mt in range(8):
                            nc.vector.tensor_tensor(Vt[:, mt, :], v16[:, mt, :],
                                                    w2bc[:], AL.mult)
                        for ns in range(4):
                            for mt in range(8):
                                lhs = E[:, mt, g, ns * 128:(ns + 1) * 128]
                                first = (g == 0 and mt == 0)
                                last = (g == HEADS - 1 and mt == 7)
                                nc.tensor.matmul(psOa[ns][:], lhs, Vt[:, mt, 0:512],
                                                 start=first, stop=last)
                                nc.tensor.matmul(psOb[ns][:], lhs, Vt[:, mt, 512:768],
                                                 start=first, stop=last)
                    if DEBUG:
                        dbgw = vtp2.tile([128, 4, DIM], f16, tag="vt2")
                        for j in range(4):
                            nc.scalar.copy(dbgw[:, j, 0:512], psOa[j][:])
                            nc.scalar.copy(dbgw[:, j, 512:768], psOb[j][:])
                        nc.sync.dma_start(dbg_w1o[:], dbgw[:])
                    for j in range(4):
                        build_u(j, psOa[j][:], psOb[j][:])
                    if DEBUG:
                        nc.sync.dma_start(dbg_u[:], u16[:])
                        nc.sync.dma_start(dbg_b2v[:], b2v[:])
                        nc.sync.dma_start(dbg_ucb[:], ucb[:])

                # + b2[g'] * colsum(u)
                with tc.tile_pool(name="psuv", bufs=1, space="PSUM") as psuv:
                    psUa = psuv.tile([128, 512], f32)
                    psUb = psuv.tile([128, 256], f32)
                    for i, j in enumerate([4, 5, 6, 7, 0, 1, 2, 3]):
                        nc.tensor.matmul(psUa[:], ones128[:], u16[:, j, 0:512],
                                         start=(i == 0), stop=(i == 7))
                        nc.tensor.matmul(psUb[:], ones128[:], u16[:, j, 512:768],
                                         start=(i == 0), stop=(i == 7))
                    nc.vector.tensor_tensor(b2u[:, 0:512], psUa[:], b2bc[:, 0:512],
                                            AL.mult)
                    nc.vector.tensor_tensor(b2u[:, 512:768], psUb[:], b2bc[:, 512:768],
                                            AL.mult)

                # ---- pass 2: out = attn3[own rows] @ u ----------------------
                with tc.tile_pool(name="pse", bufs=1, space="PSUM") as pse:
                    psPa = [pse.tile([128, 512], f32, name=f"psPa{i}") for i in range(4)]
                    psPb = [pse.tile([128, 256], f32, name=f"psPb{i}") for i in range(4)]
                    for g in range(HEADS):
                        w2bc = load_w2bc(g)
                        Ut = vtp2.tile([128, 8, DIM], f16, tag="vt2")
                        for mt in range(8):
                            nc.vector.tensor_tensor(Ut[:, mt, :], u16[:, mt, :],
                                                    w2bc[:], AL.mult)
                        for ns in range(4):
                            for mt in range(8):
                                lhs = E[:, mt, g, ns * 128:(ns + 1) * 128]
                                first = (g == 0 and mt == 0)
                                last = (g == HEADS - 1 and mt == 7)
                                nc.tensor.matmul(psPa[ns][:], lhs, Ut[:, mt, 0:512],
                                                 start=first, stop=last)
                                nc.tensor.matmul(psPb[ns][:], lhs, Ut[:, mt, 512:768],
                                                 start=first, stop=last)
                    for ns in range(4):
                        nc.vector.tensor_tensor(acc[:, ns, 0:512], psPa[ns][:],
                                                b2u[:, 0:512], AL.add)
                        nc.vector.tensor_tensor(acc[:, ns, 512:768], psPb[ns][:],
                                                b2u[:, 512:768], AL.add)

                if DEBUG:
                    nc.sync.dma_start(dbg_acc[:], acc[:])

                # ---- Phase F: output projection -----------------------------
                with tc.tile_pool(name="phf", bufs=1) as pf, \
                     tc.tile_pool(name="ypool", bufs=2) as ypool:
                    ident = pf.tile([128, 128], f32)
                    make_identity(nc, ident[:])
                    obbc = pf.tile([128, DIM], f16)
                    nc.gpsimd.dma_start(obbc[:], ob[0:1, :].to_broadcast((128, DIM)))
                    wo_sb = pf.tile([128, 6, DIM], f16)
                    nc.sync.dma_start(wo_sb[:], wo_T.rearrange("(c p) n -> p c n", p=128))
                    outT = pf.tile([128, 6, NH], f16)
                    with tc.tile_pool(name="psft", bufs=4, space="PSUM") as psft:
                        for ns in range(4):
                            for jc in range(6):
                                psT = psft.tile([128, 128], f32, tag="psT")
                                nc.tensor.transpose(psT[:],
                                                    acc[:, ns, jc * 128:(jc + 1) * 128],
                                                    ident[:])
                                nc.scalar.copy(
                                    outT[:, jc, ns * 128:(ns + 1) * 128], psT[:])
                    yr = y.rearrange("(ns p) j -> p ns j", p=128)
                    with tc.tile_pool(name="psf", bufs=2, space="PSUM") as psf:
                        for ns in range(4):
                            psY = psf.tile([128, 512], f32, tag="psY")
                            psY2 = psf.tile([128, 512], f32, tag="psY2")
                            for jc in range(6):
                                nc.tensor.matmul(psY[:, :],
                                                 outT[:, jc, ns * 128:(ns + 1) * 128],
                                                 wo_sb[:, jc, 0:512], start=(jc == 0),
                                                 stop=(jc == 5))
                                nc.tensor.matmul(psY2[:, 0:256],
                                                 outT[:, jc, ns * 128:(ns + 1) * 128],
                                                 wo_sb[:, jc, 512:768], start=(jc == 0),
                                                 stop=(jc == 5))
                            y_sb = ypool.tile([128, DIM], f32, tag="ysb")
                            nc.vector.tensor_tensor(y_sb[:, 0:512], psY[:, :],
                                                    obbc[:, 0:512], AL.add)
                            nc.vector.tensor_tensor(y_sb[:, 512:768], psY2[:, 0:256],
                                                    obbc[:, 512:768], AL.add)
                            nc.sync.dma_start(yr[:, ns, :], y_sb[:])

    nc.compile()
    return nc


def kernel(x, qkv_w, proj_l_w, proj_l_b, proj_w_w, proj_w_b, lamb,
           proj_out_w, proj_out_b):
    x = np.asarray(x, dtype=np.float32)
    qkv_w = np.asarray(qkv_w, dtype=np.float32)
    proj_l_w = np.asarray(proj_l_w, dtype=np.float32)
    proj_l_b = np.asarray(proj_l_b, dtype=np.float32)
    proj_w_w = np.asarray(proj_w_w, dtype=np.float32)
    proj_w_b = np.asarray(proj_w_b, dtype=np.float32)
    lamb = np.asarray(lamb, dtype=np.float32)
    proj_out_w = np.asarray(proj_out_w, dtype=np.float32)
    proj_out_b = np.asarray(proj_out_b, dtype=np.float32)

    nc = _build()

    wq_T = np.ascontiguousarray(qkv_w[:DIM].T * np.float32(SCALE)).astype(np.float16)
    wk_T = np.ascontiguousarray(qkv_w[DIM:2 * DIM].T).astype(np.float16)
    wv_T = np.ascontiguousarray(qkv_w[2 * DIM:].T).astype(np.float16)
    wo_T = np.ascontiguousarray(proj_out_w.T).astype(np.float16)

    w1v = np.empty((128, 72), dtype=np.float32)
    for g in range(HEADS):
        for i in range(6):
            w1v[:64, g * 6 + i] = proj_l_w[g, 2 * i]
            w1v[64:, g * 6 + i] = proj_l_w[g, 2 * i + 1]
    b1bc = np.tile(proj_l_b[None, :], (128, 1)).astype(np.float32)
    # w2f[0, g*768 + g'*64 + d] = proj_w_w[g', g]
    w2f = np.repeat(proj_w_w.T, HD, axis=1).reshape(1, HEADS * DIM).astype(np.float16)
    uc1 = np.repeat(1.0 - 2.0 * lamb, HD)[None, :].astype(np.float16)
    uc2 = np.repeat(3.0 * lamb, HD)[None, :].astype(np.float16)
    b2blk = np.repeat(proj_w_b, HD)[None, :].astype(np.float16)
    ob = proj_out_b[None, :].astype(np.float16)

    in_maps = []
    for c in range(8):
        b, half = c // 2, c % 2
        # m-axis rotated: rows [0:512] are this core's own query rows
        xr = np.concatenate([x[b, half * NH:(half + 1) * NH, :],
                             x[b, (1 - half) * NH:(2 - half) * NH, :]], axis=0)
        in_maps.append({
            "xf_T": np.ascontiguousarray(xr.T).astype(np.float16),
            "wq_T": wq_T, "wk_T": wk_T, "wv_T": wv_T, "wo_T": wo_T,
            "w1v": w1v, "b1bc": b1bc, "w2f": w2f,
            "uc1": uc1, "uc2": uc2, "b2blk": b2blk, "ob": ob,
        })

    res = run_bass_kernel_spmd(nc, in_maps, core_ids=list(range(8)),
                               trace=TRACE, **TRACE_KW)
    kernel.last_results = res
    kernel.last_nc = nc
    kernel.last_in_maps = in_maps

    out = np.empty((B, N, DIM), dtype=np.float32)
    for c in range(8):
        b, half = c // 2, c % 2
        out[b, half * NH:(half + 1) * NH, :] = res.results[c]["y"]
    return out
